# revision 6
# baseline (speedup 1.0000x reference)
"""HMM likelihood loss (forward algorithm) on 8 Trainium2 NeuronCores.

Strategy (data-parallel over batch, per sharding hint):
  - Host: log-softmax-normalize params; rewrite the forward recurrence in
    *linear* space with per-emission mean-log normalization so the scaled
    probabilities p_hat stay O(1) for the whole sequence:
        p_hat_t = (M^T p_hat_{t-1}) * That[:, obs_t]
    where M = exp(log_softmax(trans)) (row-stochastic, preserves mass) and
    That[s,e] = exp(L[s,e] - mean_s L[s,e]) (mean log factor == 0 per step).
    The exactly-known correction D[b] = sum_t mean_s L[s, obs[b,t]] is added
    back on the host at the end.
  - Host gathers the per-(t,b) emission columns into a bf16 stream (this is
    input prep for the device kernel; obs indices never need to hit HW).
  - Device (per core, batch shard of 32): 4095 serial steps, each step is
    one PE matmul [64x64]@[64x32] + one DVE elementwise multiply, with the
    emission stream DMA'd in chunks well ahead of compute.
  - Host: logp[b] = log(sum_j p_hat_T[j,b]) + D[b]; loss = -mean(logp).
"""

import sys

if "/opt/trn_rl_repo" not in sys.path:
    sys.path.insert(0, "/opt/trn_rl_repo")

from contextlib import ExitStack

import ml_dtypes
import numpy as np

import concourse.bass as bass
import concourse.tile as tile
from concourse import bacc, mybir
from concourse.alu_op_type import AluOpType
from concourse.bass_utils import run_bass_kernel_spmd

N_CORES = 8
S = 64
E = 1024
B = 256
T = 4096
BL = B // N_CORES  # 32 batch elements per core

# On-chip packing: the 32-batch is split as 2 half-batches of 16 stacked on
# the partition axis (partitions 0-63 = states x batch[0:16], 64-127 =
# states x batch[16:32]) with block-diag(M, M) weights, so every op uses the
# full 128 partitions / PE array and weight loads qualify for FWL.
P = 2 * S   # 128 partitions
HB = BL // 2  # 16 free-dim columns

NSTEPS = T - 1  # 4095 recurrence steps (step 0 folded into p0 on host)
CHUNK = 91      # emission-stream steps per DMA chunk
NCHUNK = NSTEPS // CHUNK  # 45

_BF16 = mybir.dt.bfloat16
_F32 = mybir.dt.float32


def build_nc(nsteps: int = NSTEPS, chunk: int = CHUNK, repeat: int | None = None):
    """Build the per-core Bass program (same program on all 8 cores).

    repeat: when set, wrap the whole scan in an on-device For_i loop that
    re-runs it `repeat` times (used only for HW-time measurement by diffing
    two repeat counts; the production kernel uses repeat=None)."""
    assert nsteps % chunk == 0
    nchunk = nsteps // chunk

    nc = bacc.Bacc("TRN2")
    mexp_d = nc.dram_tensor("mexp", [P, P], _BF16, kind="ExternalInput")
    p0_d = nc.dram_tensor("p0", [P, HB], _BF16, kind="ExternalInput")
    em_d = nc.dram_tensor("emits", [P, nsteps * HB], _BF16, kind="ExternalInput")
    out_d = nc.dram_tensor("pout", [P, HB], _F32, kind="ExternalOutput")

    with ExitStack() as ctx:
        tc = ctx.enter_context(tile.TileContext(nc))
        const_pool = ctx.enter_context(tc.tile_pool(name="const", bufs=1))
        p_pool = ctx.enter_context(tc.tile_pool(name="p", bufs=3))
        psum_pool = ctx.enter_context(tc.tile_pool(name="psum", bufs=2, space="PSUM"))
        em_pool = ctx.enter_context(tc.tile_pool(name="em", bufs=3))

        mexp = const_pool.tile([P, P], _BF16)
        nc.sync.dma_start(mexp[:], mexp_d.ap())

        def body():
            p = p_pool.tile([P, HB], _BF16, tag="p")
            nc.sync.dma_start(p[:], p0_d.ap())

            for c in range(nchunk):
                em = em_pool.tile([P, chunk * HB], _BF16, tag="em")
                nc.sync.dma_start(
                    em[:], em_d.ap()[:, c * chunk * HB : (c + 1) * chunk * HB]
                )
                for k in range(chunk):
                    q = psum_pool.tile([P, HB], _F32, tag="q")
                    nc.tensor.matmul(q[:], mexp[:], p[:], start=True, stop=True)
                    p2 = p_pool.tile([P, HB], _BF16, tag="p")
                    nc.vector.tensor_tensor(
                        p2[:], q[:], em[:, k * HB : (k + 1) * HB], AluOpType.mult
                    )
                    p = p2

            pf = p_pool.tile([P, HB], _F32, tag="pf")
            nc.scalar.copy(pf[:], p[:])
            nc.sync.dma_start(out_d.ap(), pf[:])

        if repeat is None:
            body()
        else:
            with tc.For_i(0, repeat, 1):
                body()

    nc.compile()
    return nc


def _log_softmax(x: np.ndarray, axis: int = -1) -> np.ndarray:
    m = np.max(x, axis=axis, keepdims=True)
    y = x - m
    return y - np.log(np.sum(np.exp(y), axis=axis, keepdims=True))


def host_prep(observations, log_initial, log_transitions, log_emissions):
    """Compute per-core device inputs + the exact host-side correction D[b]."""
    obs = np.asarray(observations)
    li = np.asarray(log_initial, np.float64)
    lt = np.asarray(log_transitions, np.float64)
    le = np.asarray(log_emissions, np.float64)

    LI = _log_softmax(li, axis=-1)                 # [S]
    M = np.exp(_log_softmax(lt, axis=-1))          # [S, S] row-stochastic
    L = _log_softmax(le, axis=-1)                  # [S, E]
    ebar = L.mean(axis=0)                          # [E] mean_s log emission
    That = np.exp(L - ebar[None, :])               # [S, E], mean log == 0

    # Exact per-batch correction: D[b] = sum over all T steps of ebar[obs].
    D = ebar[obs].sum(axis=1)                      # [B]

    # p0[j, b] = exp(LI[j] + L[j, obs[b,0]] - ebar[obs[b,0]])
    p0_all = np.exp(LI[:, None] + L[:, obs[:, 0]] - ebar[obs[:, 0]][None, :])  # [S, B]

    That_bf = That.astype(ml_dtypes.bfloat16)
    m2 = np.zeros((P, P), np.float64)
    m2[:S, :S] = M
    m2[S:, S:] = M
    mexp_bf = np.ascontiguousarray(m2.astype(ml_dtypes.bfloat16))

    def pack(x):
        """[S, ..., BL] -> [2*S, ..., HB] (batch halves stacked on axis 0)."""
        return np.concatenate([x[..., :HB], x[..., HB:]], axis=0)

    in_maps = []
    for c in range(N_CORES):
        bsl = slice(c * BL, (c + 1) * BL)
        obs_c = obs[bsl, 1:]                       # [BL, T-1]
        em = pack(That_bf[:, obs_c.T])             # [P, T-1, HB]
        p0c = pack(p0_all[:, bsl]).astype(ml_dtypes.bfloat16)
        in_maps.append(
            {
                "mexp": mexp_bf,
                "p0": np.ascontiguousarray(p0c),
                "emits": np.ascontiguousarray(em).reshape(P, NSTEPS * HB),
            }
        )
    return in_maps, D


def finish(pouts, D):
    """pouts: list of per-core [P, HB] f32 (packed) -> scalar loss."""
    cols = []
    for pp in pouts:
        pp = np.asarray(pp, np.float64)            # [P, HB]
        cols.append(np.concatenate([pp[:S, :], pp[S:, :]], axis=1))  # [S, BL]
    pT = np.concatenate(cols, axis=1)              # [S, B]
    s = pT.sum(axis=0)                             # [B]
    logp = np.log(s) + D
    return np.asarray(-logp.mean(), dtype=np.float32)


_NC_CACHE = {}


def _get_nc():
    if "nc" not in _NC_CACHE:
        _NC_CACHE["nc"] = build_nc()
    return _NC_CACHE["nc"]


def kernel(observations, log_initial, log_transitions, log_emissions):
    in_maps, D = host_prep(observations, log_initial, log_transitions, log_emissions)
    nc = _get_nc()
    res = run_bass_kernel_spmd(nc, in_maps, core_ids=list(range(N_CORES)))
    pouts = [res.results[c]["pout"] for c in range(N_CORES)]
    return finish(pouts, D)
